# revision 10
# baseline (speedup 1.0000x reference)
"""BatchHardQuadrupletLoss on Trainium2 (Bass/Tile), v3.

Same O(B^2) factoring as v1 (see kernel_baseline.py): the B^4 inter-class
tensor collapses to

    inter[a,l] = (y_pa!=y_na)(y_na!=y_l)(y_pa!=y_l)
                 * relu(hardest_pos[p_a] + m_inter - d[n_a,l])

Performance structure (TimelineSim 12680ns baseline -> v2 11125 -> v3):
 - DMA descriptors packed >=512B; E^T ships as [c0|c1|c2] + [c3] so the
   first three G chunks run while the second transfer is still in
   flight; masks (eq/ne/y-row) precomputed on host from idtys; the
   identity matrix is built on-chip from two iotas + is_equal during the
   DMA wait (bf16, which also makes the PE transpose 1 cycle/row).
 - G and the gathers run as float32r (2 cycles/row at mid-pstate; 1
   cycle/row for the N=256-padded gather rhs).  The d^2 diagonal stays
   exactly 0 (sq extracted from G itself), so sqrt cannot NaN.
 - d^2 is symmetric, so the transposed one-hots the gathers need as
   stationary operands come straight from a GPSIMD
   partition_all_reduce(max) + is_equal -- no PE transposes, no
   PSUM->SBUF staging.  The negative branch mines on -(d^2+8192*eq)
   because the gpsimd reduce has no min.
 - gather rhs is one contiguous tile [yv | hp^2 | ne | d | pad]; pu (by
   p_a) gathers [y_p | hp^2_p] (N=2) and ny (by n_a) gathers everything
   (N=256); hardest_pos is gathered squared and sqrt'd after (one [B,1]
   ACT op).  pu runs before ny so the sqrt hides under ny's PSUM-ack.
 - tail algebra: sum_l m1*relu(U'-Dn) = U*s1 - s2 with s1 = sum(m1),
   s2 = sum(m1*min(Dn-0.1,U)) -- both sums are accum_out side-outputs of
   ops already needed (m1, zz2), so the post-gather chain is two 96x96
   ops + three [B,1] ops instead of five 96x96 ops.
 - engine split: sqrt work on ACT, mining/z-chain on DVE, gpsimd on
   Pool; per-dependency ~160ns semaphore latency is hidden under queued
   work wherever the dataflow allows.

All 8 cores run the identical kernel on replicated inputs; core 0's
result is returned (the whole computation is a few us, so sharding a
scalar-output loss would only add collective latency).
"""

import numpy as np

B = 96
D = 512
NCORES = 8
MARGIN_TRIPLE = 0.2
MARGIN_INTER = 0.1
AN_OFFSET2 = 8192.0

# consts tile layout: [eq(96) | yrow(96) | yv | hp2 | ne(96) | d(96) | pad(62)]
C_EQ = 0
C_YROW = B            # 96
C_YV = 2 * B          # 192
C_HP2 = C_YV + 1      # 193
C_NE = C_YV + 2       # 194
C_D = C_NE + B        # 290
C_PAD = C_D + B       # 386
C_TOT = C_YV + 256    # 448
C_DMA = C_NE + B      # host-provided cols [0, 290)

IDENT_BF16 = False

_CACHE = {}


def _build_nc():
    import concourse.bacc as bacc
    import concourse.tile as tile
    import concourse.mybir as mybir
    from concourse import bass_isa
    from concourse.tile_rust import add_dep_helper

    def _order(after, before):
        a = getattr(after, "ins", after)
        b = getattr(before, "ins", before)
        add_dep_helper(a, b, sync=False, reason="pin DMA order")

    f32 = mybir.dt.float32
    f32r = mybir.dt.float32r
    i32 = mybir.dt.int32
    bf16 = mybir.dt.bfloat16
    AF = mybir.ActivationFunctionType
    OP = mybir.AluOpType
    AX = mybir.AxisListType

    nc = bacc.Bacc(
        "TRN2", target_bir_lowering=False, debug=False, num_devices=NCORES
    )

    h0_d = nc.dram_tensor("h0", [128, 3 * B], f32, kind="ExternalInput").ap()
    h1_d = nc.dram_tensor("h1", [128, B], f32, kind="ExternalInput").ap()
    cst_d = nc.dram_tensor("cst", [B, C_DMA], f32, kind="ExternalInput").ap()
    loss_d = nc.dram_tensor("loss", [1, 1], f32, kind="ExternalOutput").ap()

    with tile.TileContext(nc) as tc:
        with (
            tc.tile_pool(name="sb", bufs=1) as sb,
            tc.tile_pool(name="ps", bufs=1, space="PSUM") as ps,
        ):
            # ---- warmups: first ACT op is a Sqrt (single table load covers
            # sqrt/relu/identity/copy, lands during DMA); dummy matmul starts
            # the PE pstate-ramp clock ----
            dum = sb.tile([1, 1], f32)
            nc.vector.memset(dum[:], 0.0)
            dum2 = sb.tile([1, 1], f32)
            nc.scalar.activation(dum2[:], dum[:], AF.Sqrt)
            dmm = ps.tile([1, 1], f32, tag="dum")
            nc.tensor.matmul(dmm[:], dum[:], dum[:], start=True, stop=True)

            # ---- loads ----
            h0 = sb.tile([128, 3 * B], f32)
            h1 = sb.tile([128, B], f32)
            cst = sb.tile([B, C_TOT], f32)
            dma0 = nc.sync.dma_start(h0[:], h0_d)
            dma1 = nc.sync.dma_start(h1[:], h1_d)
            _order(dma1, dma0)
            dma2 = nc.sync.dma_start(cst[:, 0:C_DMA], cst_d)
            _order(dma2, dma1)

            eqm = cst[:, C_EQ : C_EQ + B]
            yrow = cst[0:1, C_YROW : C_YROW + B]

            # ---- on-chip identity (during DMA wait) + rhs pad zeroing ----
            io_r = sb.tile([B, B], f32)
            nc.gpsimd.iota(io_r[:], pattern=[[1, B]], base=0, channel_multiplier=0,
                           allow_small_or_imprecise_dtypes=True)
            io_c = sb.tile([B, 1], f32)
            nc.gpsimd.iota(io_c[:], pattern=[[1, 1]], base=0, channel_multiplier=1,
                           allow_small_or_imprecise_dtypes=True)
            ident = sb.tile([B, B], bf16 if IDENT_BF16 else f32)
            nc.vector.tensor_scalar(ident[:], io_r[:], io_c[:], None, OP.is_equal)
            nc.vector.memset(cst[:, C_PAD:C_TOT], 0.0)

            # ---- G = E @ E.T (fp32r) ----
            chunks = (h0[:, 0:B], h0[:, B : 2 * B], h0[:, 2 * B : 3 * B], h1[:])
            g = ps.tile([B, B], f32, tag="g")
            for c, ch in enumerate(chunks):
                chr_ = ch.bitcast(f32r)
                nc.tensor.matmul(g[:], chr_, chr_, start=(c == 0), stop=(c == 3))

            dmm2 = ps.tile([1, 1], f32, tag="dum")
            nc.tensor.matmul(dmm2[:], dum[:], dum[:], start=True, stop=True)

            # ---- y broadcast along free axis (host-packed row) ----
            ybs = sb.tile([B, B], f32)

            # ---- d^2 = A + A.T, A = sq_i - G (diagonal exactly 0) ----
            gsc = sb.tile([B, B], f32)
            sq = sb.tile([B, 1], f32)
            nc.vector.scalar_tensor_tensor(
                gsc[:], g[:], 1.0, ident[:], op0=OP.mult, op1=OP.mult,
                accum_out=sq[:],
            )
            av = sb.tile([B, B], f32)
            nc.vector.tensor_scalar(av[:], g[:], -1.0, sq[:], OP.mult, OP.add)
            d2 = ps.tile([B, B], f32, tag="tr")
            nc.tensor.matmul(
                d2[:], ident[:].bitcast(f32r), av[:].bitcast(f32r),
                start=True, stop=False, skip_group_check=True,
            )
            nc.tensor.matmul(
                d2[:].bitcast(f32r), av[:].bitcast(f32r), ident[:].bitcast(f32r),
                start=False, stop=True, is_transpose=True, skip_group_check=True,
            )

            # full-matrix sqrt into the gather-rhs d block (ACT)
            nc.scalar.activation(cst[:, C_D : C_D + B], d2[:], AF.Sqrt)

            # ---- batch-hard mining on d^2 (positive branch first: pu's
            # consumers are deeper than ny's) ----
            apd = sb.tile([B, B], f32)
            nc.vector.tensor_mul(apd[:], d2[:], eqm)
            anm = sb.tile([B, B], f32)
            nc.vector.scalar_tensor_tensor(
                anm[:], eqm, -AN_OFFSET2, d2[:], op0=OP.mult, op1=OP.subtract
            )
            nc.vector.tensor_reduce(
                cst[:, C_HP2 : C_HP2 + 1], apd[:], axis=AX.X, op=OP.max
            )

            mpos = sb.tile([B, B], f32)
            nc.gpsimd.partition_all_reduce(
                mpos[:], apd[:], channels=B, reduce_op=bass_isa.ReduceOp.max
            )
            mneg = sb.tile([B, B], f32)
            nc.gpsimd.partition_all_reduce(
                mneg[:], anm[:], channels=B, reduce_op=bass_isa.ReduceOp.max
            )
            nc.gpsimd.partition_broadcast(ybs[:], yrow, channels=B)

            phT = sb.tile([B, B], f32)
            nc.vector.tensor_tensor(phT[:], apd[:], mpos[:], OP.is_equal)
            nhT = sb.tile([B, B], f32)
            nc.vector.tensor_tensor(nhT[:], anm[:], mneg[:], OP.is_equal)
            hn2neg = sb.tile([B, 1], f32)
            nc.vector.tensor_reduce(hn2neg[:], anm[:], axis=AX.X, op=OP.max)

            # ---- gathers: pu first (its sqrt consumer chain is deeper) ----
            pu = ps.tile([B, 2], f32, tag="pu")
            nc.tensor.matmul(
                pu[:], phT[:].bitcast(f32r),
                cst[:, C_YV : C_YV + 2].bitcast(f32r),
                start=True, stop=True,
            )
            ny = ps.tile([B, 256], f32, tag="ny")
            nc.tensor.matmul(
                ny[:], nhT[:].bitcast(f32r),
                cst[:, C_YV:C_TOT].bitcast(f32r),
                start=True, stop=True,
            )
            nyY = ny[:, 0:1]
            nyNE = ny[:, C_NE - C_YV : C_NE - C_YV + B]
            nyD = ny[:, C_D - C_YV : C_D - C_YV + B]

            # ---- triplet branch ----
            hp_a = sb.tile([B, 1], f32)
            nc.scalar.activation(hp_a[:], cst[:, C_HP2 : C_HP2 + 1], AF.Sqrt)
            hn_a = sb.tile([B, 1], f32)
            nc.scalar.activation(hn_a[:], hn2neg[:], AF.Sqrt, scale=-1.0)
            upu = sb.tile([B, 1], f32)
            nc.scalar.activation(upu[:], pu[:, 1:2], AF.Sqrt)
            trip0 = sb.tile([B, 1], f32)
            nc.vector.scalar_tensor_tensor(
                trip0[:], hp_a[:], MARGIN_TRIPLE, hn_a[:],
                op0=OP.add, op1=OP.subtract,
            )
            tripz = sb.tile([B, 1], f32)
            nc.vector.tensor_scalar(
                tripz[:], trip0[:], 0.0, 1.0 / B, OP.max, OP.mult
            )

            # ---- inter-class tail ----
            # c1s = (y_p != y_n)/B^2; m1 = (y_l!=y_p)*ne[n,:] with s1 = sum_l;
            # t1 = min(Dn-0.1, U); s2 = sum_l m1*t1;
            # per-anchor inter mean = c1s*(U*s1 - s2)
            m1 = sb.tile([B, B], f32)
            s1 = sb.tile([B, 1], f32)
            nc.vector.scalar_tensor_tensor(
                m1[:], ybs[:], pu[:, 0:1], nyNE, op0=OP.not_equal, op1=OP.mult,
                accum_out=s1[:],
            )
            t1 = sb.tile([B, B], f32)
            nc.vector.tensor_scalar(
                t1[:], nyD, -MARGIN_INTER, upu[:], OP.add, OP.min
            )
            c1s = sb.tile([B, 1], f32)
            nc.vector.tensor_scalar(
                c1s[:], nyY, pu[:, 0:1], 1.0 / (B * B), OP.not_equal, OP.mult
            )
            zz = sb.tile([B, B], f32)
            s2 = sb.tile([B, 1], f32)
            nc.vector.scalar_tensor_tensor(
                zz[:], m1[:], 1.0, t1[:], op0=OP.mult, op1=OP.mult,
                accum_out=s2[:],
            )
            q2 = sb.tile([B, 1], f32)
            nc.vector.scalar_tensor_tensor(
                q2[:], s1[:], upu[:], s2[:], op0=OP.mult, op1=OP.subtract
            )
            comb = sb.tile([B, 1], f32)
            nc.vector.scalar_tensor_tensor(
                comb[:], q2[:], c1s[:], tripz[:], op0=OP.mult, op1=OP.add
            )

            res = sb.tile([B, 1], f32)
            nc.gpsimd.partition_all_reduce(
                res[:], comb[:], channels=B, reduce_op=bass_isa.ReduceOp.add
            )
            nc.sync.dma_start(loss_d, res[0:1, :])

    nc.compile()
    return nc


def _get_nc():
    if "nc" not in _CACHE:
        _CACHE["nc"] = _build_nc()
    return _CACHE["nc"]


def _in_map(embs, idtys):
    embs = np.asarray(embs, dtype=np.float32)
    y = np.asarray(idtys).astype(np.float32).reshape(B)
    et = np.ascontiguousarray(embs.T)  # [512, 96]

    h0 = np.empty((128, 3 * B), dtype=np.float32)
    h0[:, 0:B] = et[0:128]
    h0[:, B : 2 * B] = et[128:256]
    h0[:, 2 * B : 3 * B] = et[256:384]
    h1 = np.ascontiguousarray(et[384:512])

    eq = (y[:, None] == y[None, :]).astype(np.float32)
    cst = np.zeros((B, C_DMA), dtype=np.float32)
    cst[:, C_EQ : C_EQ + B] = eq
    cst[0, C_YROW : C_YROW + B] = y
    cst[:, C_YV] = y
    cst[:, C_NE : C_NE + B] = 1.0 - eq

    return {
        "h0": np.ascontiguousarray(h0),
        "h1": h1,
        "cst": np.ascontiguousarray(cst),
    }


def kernel(embs, idtys, **_ignored):
    from concourse.bass_utils import run_bass_kernel_spmd

    nc = _get_nc()
    in_map = _in_map(embs, idtys)
    out = run_bass_kernel_spmd(
        nc,
        [dict(in_map) for _ in range(NCORES)],
        core_ids=list(range(NCORES)),
    )
    return np.array(out.results[0]["loss"][0, 0], dtype=np.float32)


# revision 11
# speedup vs baseline: 1.0480x; 1.0480x over previous
"""BatchHardQuadrupletLoss on Trainium2 (Bass/Tile), v3.

Same O(B^2) factoring as v1 (see kernel_baseline.py): the B^4 inter-class
tensor collapses to

    inter[a,l] = (y_pa!=y_na)(y_na!=y_l)(y_pa!=y_l)
                 * relu(hardest_pos[p_a] + m_inter - d[n_a,l])

Performance structure (TimelineSim 12680ns baseline -> v2 11125 -> v3):
 - DMA descriptors packed >=512B; E^T ships as [c0|c1|c2] + [c3] so the
   first three G chunks run while the second transfer is still in
   flight; masks (eq/ne/y-row) precomputed on host from idtys; the
   identity matrix is built on-chip from two iotas + is_equal during the
   DMA wait (bf16, which also makes the PE transpose 1 cycle/row).
 - G and the gathers run as float32r (2 cycles/row at mid-pstate; 1
   cycle/row for the N=256-padded gather rhs).  The d^2 diagonal stays
   exactly 0 (sq extracted from G itself), so sqrt cannot NaN.
 - d^2 is symmetric, so the transposed one-hots the gathers need as
   stationary operands come straight from a GPSIMD
   partition_all_reduce(max) + is_equal -- no PE transposes, no
   PSUM->SBUF staging.  The negative branch mines on -(d^2+8192*eq)
   because the gpsimd reduce has no min.
 - gather rhs is one contiguous tile [yv | hp^2 | ne | d | pad]; pu (by
   p_a) gathers [y_p | hp^2_p] (N=2) and ny (by n_a) gathers everything
   (N=256); hardest_pos is gathered squared and sqrt'd after (one [B,1]
   ACT op).  pu runs before ny so the sqrt hides under ny's PSUM-ack.
 - tail algebra: sum_l m1*relu(U'-Dn) = U*s1 - s2 with s1 = sum(m1),
   s2 = sum(m1*min(Dn-0.1,U)) -- both sums are accum_out side-outputs of
   ops already needed (m1, zz2), so the post-gather chain is two 96x96
   ops + three [B,1] ops instead of five 96x96 ops.
 - engine split: sqrt work on ACT, mining/z-chain on DVE, gpsimd on
   Pool; per-dependency ~160ns semaphore latency is hidden under queued
   work wherever the dataflow allows.

All 8 cores run the identical kernel on replicated inputs; core 0's
result is returned (the whole computation is a few us, so sharding a
scalar-output loss would only add collective latency).
"""

import numpy as np

B = 96
D = 512
NCORES = 8
MARGIN_TRIPLE = 0.2
MARGIN_INTER = 0.1
AN_OFFSET2 = 8192.0

# consts tile layout: [eq(96) | yrow(96) | yv | hp2 | ne(96) | d(96) | pad(62)]
C_EQ = 0
C_YROW = B            # 96
C_YV = 2 * B          # 192
C_HP2 = C_YV + 1      # 193
C_NE = C_YV + 2       # 194
C_D = C_NE + B        # 290
C_PAD = C_D + B       # 386
C_TOT = C_YV + 256    # 448
C_DMA = C_NE + B      # host-provided cols [0, 290)

IDENT_BF16 = False

_CACHE = {}


def _build_nc():
    import concourse.bacc as bacc
    import concourse.tile as tile
    import concourse.mybir as mybir
    from concourse import bass_isa
    from concourse.tile_rust import add_dep_helper

    def _order(after, before):
        a = getattr(after, "ins", after)
        b = getattr(before, "ins", before)
        add_dep_helper(a, b, sync=False, reason="pin DMA order")

    f32 = mybir.dt.float32
    f32r = mybir.dt.float32r
    i32 = mybir.dt.int32
    bf16 = mybir.dt.bfloat16
    AF = mybir.ActivationFunctionType
    OP = mybir.AluOpType
    AX = mybir.AxisListType

    nc = bacc.Bacc(
        "TRN2", target_bir_lowering=False, debug=False, num_devices=NCORES
    )

    h0_d = nc.dram_tensor("h0", [128, 3 * B], f32, kind="ExternalInput").ap()
    h1_d = nc.dram_tensor("h1", [128, B], f32, kind="ExternalInput").ap()
    cst_d = nc.dram_tensor("cst", [B, C_DMA], f32, kind="ExternalInput").ap()
    loss_d = nc.dram_tensor("loss", [1, 1], f32, kind="ExternalOutput").ap()

    with tile.TileContext(nc) as tc:
        with (
            tc.tile_pool(name="sb", bufs=1) as sb,
            tc.tile_pool(name="ps", bufs=1, space="PSUM") as ps,
        ):
            # ---- warmups: first ACT op is a Sqrt (single table load covers
            # sqrt/relu/identity/copy, lands during DMA); dummy matmul starts
            # the PE pstate-ramp clock ----
            dum = sb.tile([1, 1], f32)
            nc.vector.memset(dum[:], 0.0)
            dum2 = sb.tile([1, 1], f32)
            nc.scalar.activation(dum2[:], dum[:], AF.Sqrt)
            dmm = ps.tile([1, 1], f32, tag="dum")
            nc.tensor.matmul(dmm[:], dum[:], dum[:], start=True, stop=True)

            # ---- loads ----
            h0 = sb.tile([128, 3 * B], f32)
            h1 = sb.tile([128, B], f32)
            cst = sb.tile([B, C_TOT], f32)
            dma0 = nc.sync.dma_start(h0[:], h0_d)
            dma1 = nc.sync.dma_start(h1[:], h1_d)
            _order(dma1, dma0)
            dma2 = nc.sync.dma_start(cst[:, 0:C_DMA], cst_d)
            _order(dma2, dma1)

            eqm = cst[:, C_EQ : C_EQ + B]
            yrow = cst[0:1, C_YROW : C_YROW + B]

            # ---- on-chip identity (during DMA wait) + rhs pad zeroing ----
            io_r = sb.tile([B, B], f32)
            nc.gpsimd.iota(io_r[:], pattern=[[1, B]], base=0, channel_multiplier=0,
                           allow_small_or_imprecise_dtypes=True)
            io_c = sb.tile([B, 1], f32)
            nc.gpsimd.iota(io_c[:], pattern=[[1, 1]], base=0, channel_multiplier=1,
                           allow_small_or_imprecise_dtypes=True)
            ident = sb.tile([B, B], bf16 if IDENT_BF16 else f32)
            nc.vector.tensor_scalar(ident[:], io_r[:], io_c[:], None, OP.is_equal)
            nc.vector.memset(cst[:, C_PAD:C_TOT], 0.0)

            # ---- G = E @ E.T (fp32r) ----
            chunks = (h0[:, 0:B], h0[:, B : 2 * B], h0[:, 2 * B : 3 * B], h1[:])
            g = ps.tile([B, B], f32, tag="g")
            for c, ch in enumerate(chunks):
                chr_ = ch.bitcast(f32r)
                nc.tensor.matmul(g[:], chr_, chr_, start=(c == 0), stop=(c == 3))

            dmm2 = ps.tile([1, 1], f32, tag="dum")
            nc.tensor.matmul(dmm2[:], dum[:], dum[:], start=True, stop=True)

            # ---- y broadcast along free axis (host-packed row) ----
            ybs = sb.tile([B, B], f32)

            # ---- d^2 = A + A.T, A = sq_i - G (diagonal exactly 0) ----
            gsc = sb.tile([B, B], f32)
            sq = sb.tile([B, 1], f32)
            nc.vector.scalar_tensor_tensor(
                gsc[:], g[:], 1.0, ident[:], op0=OP.mult, op1=OP.mult,
                accum_out=sq[:],
            )
            av = sb.tile([B, B], f32)
            nc.vector.tensor_scalar(av[:], g[:], -1.0, sq[:], OP.mult, OP.add)
            avt = ps.tile([B, B], f32, tag="tr")
            nc.tensor.transpose(avt[:].bitcast(f32r), av[:].bitcast(f32r), ident[:].bitcast(f32r))
            d2 = sb.tile([B, B], f32)
            nc.vector.tensor_add(d2[:], av[:], avt[:])

            # full-matrix sqrt into the gather-rhs d block (ACT)
            nc.scalar.activation(cst[:, C_D : C_D + B], d2[:], AF.Sqrt)

            # ---- batch-hard mining on d^2 (positive branch first: pu's
            # consumers are deeper than ny's) ----
            apd = sb.tile([B, B], f32)
            nc.vector.tensor_mul(apd[:], d2[:], eqm)
            anm = sb.tile([B, B], f32)
            nc.vector.scalar_tensor_tensor(
                anm[:], eqm, -AN_OFFSET2, d2[:], op0=OP.mult, op1=OP.subtract
            )
            nc.vector.tensor_reduce(
                cst[:, C_HP2 : C_HP2 + 1], apd[:], axis=AX.X, op=OP.max
            )

            mpos = sb.tile([B, B], f32)
            nc.gpsimd.partition_all_reduce(
                mpos[:], apd[:], channels=B, reduce_op=bass_isa.ReduceOp.max
            )
            mneg = sb.tile([B, B], f32)
            nc.gpsimd.partition_all_reduce(
                mneg[:], anm[:], channels=B, reduce_op=bass_isa.ReduceOp.max
            )
            nc.gpsimd.partition_broadcast(ybs[:], yrow, channels=B)

            phT = sb.tile([B, B], f32)
            nc.vector.tensor_tensor(phT[:], apd[:], mpos[:], OP.is_equal)
            nhT = sb.tile([B, B], f32)
            nc.vector.tensor_tensor(nhT[:], anm[:], mneg[:], OP.is_equal)
            hn2neg = sb.tile([B, 1], f32)
            nc.vector.tensor_reduce(hn2neg[:], anm[:], axis=AX.X, op=OP.max)

            # ---- gathers: pu first (its sqrt consumer chain is deeper) ----
            pu = ps.tile([B, 2], f32, tag="pu")
            nc.tensor.matmul(
                pu[:], phT[:].bitcast(f32r),
                cst[:, C_YV : C_YV + 2].bitcast(f32r),
                start=True, stop=True,
            )
            ny = ps.tile([B, 256], f32, tag="ny")
            nc.tensor.matmul(
                ny[:], nhT[:].bitcast(f32r),
                cst[:, C_YV:C_TOT].bitcast(f32r),
                start=True, stop=True,
            )
            nyY = ny[:, 0:1]
            nyNE = ny[:, C_NE - C_YV : C_NE - C_YV + B]
            nyD = ny[:, C_D - C_YV : C_D - C_YV + B]

            # ---- triplet branch ----
            hp_a = sb.tile([B, 1], f32)
            nc.scalar.activation(hp_a[:], cst[:, C_HP2 : C_HP2 + 1], AF.Sqrt)
            hn_a = sb.tile([B, 1], f32)
            nc.scalar.activation(hn_a[:], hn2neg[:], AF.Sqrt, scale=-1.0)
            upu = sb.tile([B, 1], f32)
            nc.scalar.activation(upu[:], pu[:, 1:2], AF.Sqrt)
            trip0 = sb.tile([B, 1], f32)
            nc.vector.scalar_tensor_tensor(
                trip0[:], hp_a[:], MARGIN_TRIPLE, hn_a[:],
                op0=OP.add, op1=OP.subtract,
            )
            tripz = sb.tile([B, 1], f32)
            nc.vector.tensor_scalar(
                tripz[:], trip0[:], 0.0, 1.0 / B, OP.max, OP.mult
            )

            # ---- inter-class tail ----
            # c1s = (y_p != y_n)/B^2; m1 = (y_l!=y_p)*ne[n,:] with s1 = sum_l;
            # t1 = min(Dn-0.1, U); s2 = sum_l m1*t1;
            # per-anchor inter mean = c1s*(U*s1 - s2)
            m1 = sb.tile([B, B], f32)
            s1 = sb.tile([B, 1], f32)
            nc.vector.scalar_tensor_tensor(
                m1[:], ybs[:], pu[:, 0:1], nyNE, op0=OP.not_equal, op1=OP.mult,
                accum_out=s1[:],
            )
            t1 = sb.tile([B, B], f32)
            nc.vector.tensor_scalar(
                t1[:], nyD, -MARGIN_INTER, upu[:], OP.add, OP.min
            )
            c1s = sb.tile([B, 1], f32)
            nc.vector.tensor_scalar(
                c1s[:], nyY, pu[:, 0:1], 1.0 / (B * B), OP.not_equal, OP.mult
            )
            zz = sb.tile([B, B], f32)
            s2 = sb.tile([B, 1], f32)
            nc.vector.scalar_tensor_tensor(
                zz[:], m1[:], 1.0, t1[:], op0=OP.mult, op1=OP.mult,
                accum_out=s2[:],
            )
            q2 = sb.tile([B, 1], f32)
            nc.vector.scalar_tensor_tensor(
                q2[:], s1[:], upu[:], s2[:], op0=OP.mult, op1=OP.subtract
            )
            comb = sb.tile([B, 1], f32)
            nc.vector.scalar_tensor_tensor(
                comb[:], q2[:], c1s[:], tripz[:], op0=OP.mult, op1=OP.add
            )

            res = sb.tile([B, 1], f32)
            nc.gpsimd.partition_all_reduce(
                res[:], comb[:], channels=B, reduce_op=bass_isa.ReduceOp.add
            )
            nc.sync.dma_start(loss_d, res[0:1, :])

    nc.compile()
    return nc


def _get_nc():
    if "nc" not in _CACHE:
        _CACHE["nc"] = _build_nc()
    return _CACHE["nc"]


def _in_map(embs, idtys):
    embs = np.asarray(embs, dtype=np.float32)
    y = np.asarray(idtys).astype(np.float32).reshape(B)
    et = np.ascontiguousarray(embs.T)  # [512, 96]

    h0 = np.empty((128, 3 * B), dtype=np.float32)
    h0[:, 0:B] = et[0:128]
    h0[:, B : 2 * B] = et[128:256]
    h0[:, 2 * B : 3 * B] = et[256:384]
    h1 = np.ascontiguousarray(et[384:512])

    eq = (y[:, None] == y[None, :]).astype(np.float32)
    cst = np.zeros((B, C_DMA), dtype=np.float32)
    cst[:, C_EQ : C_EQ + B] = eq
    cst[0, C_YROW : C_YROW + B] = y
    cst[:, C_YV] = y
    cst[:, C_NE : C_NE + B] = 1.0 - eq

    return {
        "h0": np.ascontiguousarray(h0),
        "h1": h1,
        "cst": np.ascontiguousarray(cst),
    }


def kernel(embs, idtys, **_ignored):
    from concourse.bass_utils import run_bass_kernel_spmd

    nc = _get_nc()
    in_map = _in_map(embs, idtys)
    out = run_bass_kernel_spmd(
        nc,
        [dict(in_map) for _ in range(NCORES)],
        core_ids=list(range(NCORES)),
    )
    return np.array(out.results[0]["loss"][0, 0], dtype=np.float32)


# revision 12
# speedup vs baseline: 1.0525x; 1.0044x over previous
"""BatchHardQuadrupletLoss on Trainium2 (Bass/Tile), v3.

Same O(B^2) factoring as v1 (see kernel_baseline.py): the B^4 inter-class
tensor collapses to

    inter[a,l] = (y_pa!=y_na)(y_na!=y_l)(y_pa!=y_l)
                 * relu(hardest_pos[p_a] + m_inter - d[n_a,l])

Performance structure (TimelineSim 12680ns baseline -> v2 11125 -> v3):
 - DMA descriptors packed >=512B; E^T ships as [c0|c1|c2] + [c3] so the
   first three G chunks run while the second transfer is still in
   flight; masks (eq/ne/y-row) precomputed on host from idtys; the
   identity matrix is built on-chip from two iotas + is_equal during the
   DMA wait (bf16, which also makes the PE transpose 1 cycle/row).
 - G and the gathers run as float32r (2 cycles/row at mid-pstate; 1
   cycle/row for the N=256-padded gather rhs).  The d^2 diagonal stays
   exactly 0 (sq extracted from G itself), so sqrt cannot NaN.
 - d^2 is symmetric, so the transposed one-hots the gathers need as
   stationary operands come straight from a GPSIMD
   partition_all_reduce(max) + is_equal -- no PE transposes, no
   PSUM->SBUF staging.  The negative branch mines on -(d^2+8192*eq)
   because the gpsimd reduce has no min.
 - gather rhs is one contiguous tile [yv | hp^2 | ne | d | pad]; pu (by
   p_a) gathers [y_p | hp^2_p] (N=2) and ny (by n_a) gathers everything
   (N=256); hardest_pos is gathered squared and sqrt'd after (one [B,1]
   ACT op).  pu runs before ny so the sqrt hides under ny's PSUM-ack.
 - tail algebra: sum_l m1*relu(U'-Dn) = U*s1 - s2 with s1 = sum(m1),
   s2 = sum(m1*min(Dn-0.1,U)) -- both sums are accum_out side-outputs of
   ops already needed (m1, zz2), so the post-gather chain is two 96x96
   ops + three [B,1] ops instead of five 96x96 ops.
 - engine split: sqrt work on ACT, mining/z-chain on DVE, gpsimd on
   Pool; per-dependency ~160ns semaphore latency is hidden under queued
   work wherever the dataflow allows.

All 8 cores run the identical kernel on replicated inputs; core 0's
result is returned (the whole computation is a few us, so sharding a
scalar-output loss would only add collective latency).
"""

import numpy as np

B = 96
D = 512
NCORES = 8
MARGIN_TRIPLE = 0.2
MARGIN_INTER = 0.1
AN_OFFSET2 = 8192.0

# consts tile layout: [eq(96) | yrow(96) | yv | hp2 | ne(96) | d(96) | pad(62)]
C_EQ = 0
C_YROW = B            # 96
C_YV = 2 * B          # 192
C_HP2 = C_YV + 1      # 193
C_NE = C_YV + 2       # 194
C_D = C_NE + B        # 290
C_PAD = C_D + B       # 386
C_TOT = C_YV + 256    # 448
C_DMA = C_NE + B      # host-provided cols [0, 290)

IDENT_BF16 = False

_CACHE = {}


def _build_nc():
    import concourse.bacc as bacc
    import concourse.tile as tile
    import concourse.mybir as mybir
    from concourse import bass_isa
    from concourse.tile_rust import add_dep_helper

    def _order(after, before):
        a = getattr(after, "ins", after)
        b = getattr(before, "ins", before)
        add_dep_helper(a, b, sync=False, reason="pin DMA order")

    f32 = mybir.dt.float32
    f32r = mybir.dt.float32r
    i32 = mybir.dt.int32
    bf16 = mybir.dt.bfloat16
    AF = mybir.ActivationFunctionType
    OP = mybir.AluOpType
    AX = mybir.AxisListType

    nc = bacc.Bacc(
        "TRN2", target_bir_lowering=False, debug=False, num_devices=NCORES
    )

    h0_d = nc.dram_tensor("h0", [128, 3 * B], f32, kind="ExternalInput").ap()
    h1_d = nc.dram_tensor("h1", [128, B], f32, kind="ExternalInput").ap()
    cst_d = nc.dram_tensor("cst", [B, C_DMA], f32, kind="ExternalInput").ap()
    loss_d = nc.dram_tensor("loss", [1, 1], f32, kind="ExternalOutput").ap()

    with tile.TileContext(nc) as tc:
        with (
            tc.tile_pool(name="sb", bufs=1) as sb,
            tc.tile_pool(name="ps", bufs=1, space="PSUM") as ps,
        ):
            # ---- warmups: first ACT op is a Sqrt (single table load covers
            # sqrt/relu/identity/copy, lands during DMA); dummy matmul starts
            # the PE pstate-ramp clock ----
            dum = sb.tile([1, 1], f32)
            nc.vector.memset(dum[:], 0.0)
            dum2 = sb.tile([1, 1], f32)
            nc.scalar.activation(dum2[:], dum[:], AF.Sqrt)
            dmm = ps.tile([1, 1], f32, tag="dum")
            nc.tensor.matmul(dmm[:], dum[:], dum[:], start=True, stop=True)

            # ---- loads ----
            h0 = sb.tile([128, 3 * B], f32)
            h1 = sb.tile([128, B], f32)
            cst = sb.tile([B, C_TOT], f32)
            dma0 = nc.sync.dma_start(h0[:], h0_d)
            dma1 = nc.sync.dma_start(h1[:], h1_d)
            _order(dma1, dma0)
            dma2 = nc.sync.dma_start(cst[:, 0:C_DMA], cst_d)
            _order(dma2, dma1)

            eqm = cst[:, C_EQ : C_EQ + B]
            yrow = cst[0:1, C_YROW : C_YROW + B]

            # ---- on-chip identity (during DMA wait) + rhs pad zeroing ----
            io_r = sb.tile([B, B], f32)
            nc.gpsimd.iota(io_r[:], pattern=[[1, B]], base=0, channel_multiplier=0,
                           allow_small_or_imprecise_dtypes=True)
            io_c = sb.tile([B, 1], f32)
            nc.gpsimd.iota(io_c[:], pattern=[[1, 1]], base=0, channel_multiplier=1,
                           allow_small_or_imprecise_dtypes=True)
            ident = sb.tile([B, B], bf16 if IDENT_BF16 else f32)
            nc.vector.tensor_scalar(ident[:], io_r[:], io_c[:], None, OP.is_equal)
            nc.vector.memset(cst[:, C_PAD:C_TOT], 0.0)

            # ---- G = E @ E.T (fp32r) ----
            chunks = (h0[:, 0:B], h0[:, B : 2 * B], h0[:, 2 * B : 3 * B], h1[:])
            g = ps.tile([B, B], f32, tag="g")
            for c, ch in enumerate(chunks):
                chr_ = ch.bitcast(f32r)
                nc.tensor.matmul(g[:], chr_, chr_, start=(c == 0), stop=(c == 3))

            dmm2 = ps.tile([1, 1], f32, tag="dum")
            nc.tensor.matmul(dmm2[:], dum[:], dum[:], start=True, stop=True)

            # ---- y broadcast along free axis (host-packed row) ----
            ybs = sb.tile([B, B], f32)

            # ---- d^2 = A + A.T, A = sq_i - G (diagonal exactly 0) ----
            gsc = sb.tile([B, B], f32)
            sq = sb.tile([B, 1], f32)
            nc.vector.scalar_tensor_tensor(
                gsc[:], g[:], 1.0, ident[:], op0=OP.mult, op1=OP.mult,
                accum_out=sq[:],
            )
            av = sb.tile([B, B], f32)
            nc.vector.tensor_scalar(av[:], g[:], -1.0, sq[:], OP.mult, OP.add)
            avt = ps.tile([B, B], f32, tag="tr")
            nc.tensor.transpose(avt[:].bitcast(f32r), av[:].bitcast(f32r), ident[:].bitcast(f32r))
            d2 = sb.tile([B, B], f32)
            nc.vector.tensor_add(d2[:], av[:], avt[:])

            # full-matrix sqrt into the gather-rhs d block (ACT)
            nc.scalar.activation(cst[:, C_D : C_D + B], d2[:], AF.Sqrt)

            # ---- batch-hard mining on d^2 (positive branch first: pu's
            # consumers are deeper than ny's) ----
            apd = sb.tile([B, B], f32)
            nc.vector.tensor_mul(apd[:], d2[:], eqm)
            anm = sb.tile([B, B], f32)
            nc.vector.scalar_tensor_tensor(
                anm[:], eqm, -AN_OFFSET2, d2[:], op0=OP.mult, op1=OP.subtract
            )
            nc.vector.tensor_reduce(
                cst[:, C_HP2 : C_HP2 + 1], apd[:], axis=AX.X, op=OP.max
            )

            mpos = sb.tile([B, B], f32)
            nc.gpsimd.partition_all_reduce(
                mpos[:], apd[:], channels=B, reduce_op=bass_isa.ReduceOp.max
            )
            mneg = sb.tile([B, B], f32)
            nc.gpsimd.partition_all_reduce(
                mneg[:], anm[:], channels=B, reduce_op=bass_isa.ReduceOp.max
            )
            nc.gpsimd.partition_broadcast(ybs[:], yrow, channels=B)

            phT = sb.tile([B, B], f32)
            nc.vector.tensor_tensor(phT[:], apd[:], mpos[:], OP.is_equal)
            nhT = sb.tile([B, B], f32)
            nc.vector.tensor_tensor(nhT[:], anm[:], mneg[:], OP.is_equal)
            hn2neg = sb.tile([B, 1], f32)
            nc.vector.tensor_reduce(hn2neg[:], anm[:], axis=AX.X, op=OP.max)

            # ---- gathers: pu first (its sqrt consumer chain is deeper) ----
            pu = ps.tile([B, 2], f32, tag="pu")
            nc.tensor.matmul(
                pu[:], phT[:].bitcast(f32r),
                cst[:, C_YV : C_YV + 2].bitcast(f32r),
                start=True, stop=True,
            )
            ny = ps.tile([B, 256], f32, tag="ny")
            nc.tensor.matmul(
                ny[:], nhT[:].bitcast(f32r),
                cst[:, C_YV:C_TOT].bitcast(f32r),
                start=True, stop=True,
            )
            nyY = ny[:, 0:1]
            nyNE = ny[:, C_NE - C_YV : C_NE - C_YV + B]
            nyD = ny[:, C_D - C_YV : C_D - C_YV + B]

            # ---- triplet branch ----
            hp_a = sb.tile([B, 1], f32)
            nc.scalar.activation(hp_a[:], cst[:, C_HP2 : C_HP2 + 1], AF.Sqrt)
            hn_a = sb.tile([B, 1], f32)
            nc.scalar.activation(hn_a[:], hn2neg[:], AF.Sqrt, scale=-1.0)
            upu = sb.tile([B, 1], f32)
            nc.scalar.activation(upu[:], pu[:, 1:2], AF.Sqrt)
            trip0 = sb.tile([B, 1], f32)
            nc.vector.scalar_tensor_tensor(
                trip0[:], hp_a[:], MARGIN_TRIPLE, hn_a[:],
                op0=OP.add, op1=OP.subtract,
            )
            tripz = sb.tile([B, 1], f32)
            nc.vector.tensor_scalar(
                tripz[:], trip0[:], 0.0, 1.0 / B, OP.max, OP.mult
            )

            # ---- inter-class tail ----
            # c1s = (y_p != y_n)/B^2; m1 = (y_l!=y_p)*ne[n,:] with s1 = sum_l;
            # t1 = min(Dn-0.1, U); s2 = sum_l m1*t1;
            # per-anchor inter mean = c1s*(U*s1 - s2)
            upu1 = sb.tile([B, 1], f32)
            nc.vector.tensor_scalar(upu1[:], upu[:], MARGIN_INTER, None, OP.add)
            c1s = sb.tile([B, 1], f32)
            nc.vector.tensor_scalar(
                c1s[:], nyY, pu[:, 0:1], 1.0 / (B * B), OP.not_equal, OP.mult
            )
            m1 = sb.tile([B, B], f32)
            nc.vector.scalar_tensor_tensor(
                m1[:], ybs[:], pu[:, 0:1], nyNE, op0=OP.not_equal, op1=OP.mult
            )
            s0r = sb.tile([B, B], f32)
            nc.scalar.activation(s0r[:], nyD, AF.Relu, bias=upu1[:], scale=-1.0)
            zfin = sb.tile([B, B], f32)
            isum = sb.tile([B, 1], f32)
            nc.vector.scalar_tensor_tensor(
                zfin[:], m1[:], c1s[:], s0r[:], op0=OP.mult, op1=OP.mult,
                accum_out=isum[:],
            )
            comb = sb.tile([B, 1], f32)
            nc.vector.scalar_tensor_tensor(
                comb[:], isum[:], 1.0, tripz[:], op0=OP.mult, op1=OP.add
            )

            res = sb.tile([B, 1], f32)
            nc.gpsimd.partition_all_reduce(
                res[:], comb[:], channels=B, reduce_op=bass_isa.ReduceOp.add
            )
            nc.sync.dma_start(loss_d, res[0:1, :])

    nc.compile()
    return nc


def _get_nc():
    if "nc" not in _CACHE:
        _CACHE["nc"] = _build_nc()
    return _CACHE["nc"]


def _in_map(embs, idtys):
    embs = np.asarray(embs, dtype=np.float32)
    y = np.asarray(idtys).astype(np.float32).reshape(B)
    et = np.ascontiguousarray(embs.T)  # [512, 96]

    h0 = np.empty((128, 3 * B), dtype=np.float32)
    h0[:, 0:B] = et[0:128]
    h0[:, B : 2 * B] = et[128:256]
    h0[:, 2 * B : 3 * B] = et[256:384]
    h1 = np.ascontiguousarray(et[384:512])

    eq = (y[:, None] == y[None, :]).astype(np.float32)
    cst = np.zeros((B, C_DMA), dtype=np.float32)
    cst[:, C_EQ : C_EQ + B] = eq
    cst[0, C_YROW : C_YROW + B] = y
    cst[:, C_YV] = y
    cst[:, C_NE : C_NE + B] = 1.0 - eq

    return {
        "h0": np.ascontiguousarray(h0),
        "h1": h1,
        "cst": np.ascontiguousarray(cst),
    }


def kernel(embs, idtys, **_ignored):
    from concourse.bass_utils import run_bass_kernel_spmd

    nc = _get_nc()
    in_map = _in_map(embs, idtys)
    out = run_bass_kernel_spmd(
        nc,
        [dict(in_map) for _ in range(NCORES)],
        core_ids=list(range(NCORES)),
    )
    return np.array(out.results[0]["loss"][0, 0], dtype=np.float32)
